# revision 1
# baseline (speedup 1.0000x reference)
"""Trainium2 Bass kernel for the CrossAttention reference module.

  claim = x[claim_index]; evidence = x[evidence_index]
  wc = claim @ Wc + bc; we = evidence @ We + be
  S = wc @ we.T + blockdiag_mask(batch[claim_index], batch[evidence_index])
  A = softmax(S, -1); cn = A @ evidence
  a = concat([claim, cn, claim-cn, claim*cn]) @ Wa + ba
  out = segment_mean(a, batch[claim_index], 64)

Sharding: claim rows (4096) are split 512 per NeuronCore across 8 cores;
evidence set and params are replicated.  Each core computes a partial
segment sum [64, 512]; the host sums the 8 partials, divides by the
per-graph claim counts, and adds ba (segment_mean(a+ba) = segment_mean(a)+ba).

Masking trick: the 64-dim projections are augmented with 64 extra one-hot
"graph id" dims scaled by 32.0, so the score matmul produces
S + 1024*same_graph.  exp(S_aug - 1074) underflows to exactly 0 for
cross-graph pairs and equals exp(S - 50) for same-graph pairs -- a
row-constant shift softmax ignores.  (Needs every claim row to have some
same-graph score > ~-37, which holds with huge margin here.)

Matmuls run in bf16 with fp32 PSUM accumulation; the final segment-sum
matmul is fp32.
"""

import sys

if "/opt/trn_rl_repo" not in sys.path:
    sys.path.insert(0, "/opt/trn_rl_repo")

import numpy as np

import concourse.bass as bass
import concourse.mybir as mybir
import concourse.tile as tile
from concourse.bass_utils import run_bass_kernel_spmd
from concourse.masks import make_identity
from concourse.vector_clock import ScopedClock

P = 128
NHID = 512
PROJ = 64
N_NODES = 16384
NC_ALL = 4096
NE = 8192
NG = 64
N_CORES = 8
NC_LOC = NC_ALL // N_CORES  # 512 claims per core
NE_LOC = 2048               # padded per-core evidence rows (claims are sorted
                            # by graph on the host, so a core's 512 claims
                            # span ~9 graphs ~= 1.2k evidence rows)
KT_H = NHID // P            # 4 hidden k-tiles
ET = NE_LOC // P            # 16 evidence tiles per core
CT = NC_LOC // P            # 4 claim tiles per core
KO = 4 * NHID // P          # 16 k-tiles of the concat dim
NCHUNK = NE_LOC // NHID     # 4 score chunks per claim tile
MAG = 32.0                  # sqrt(1024): one-hot scale
EXP_BIAS = -(MAG * MAG + 50.0)  # exp(S + 1024 - 1074) = exp(S - 50)

f32 = mybir.dt.float32
bf16 = mybir.dt.bfloat16
i32 = mybir.dt.int32
AF = mybir.ActivationFunctionType
ALU = mybir.AluOpType


class _PatchedTileContext(tile.TileContext):
    """Workaround: this neuronxcc/walrus build rejects InstDrain carrying
    sync waits ("Too many sync wait commands").  Collect the final drain's
    waits on nops (one wait each) and emit the drain itself wait-free."""

    def _drain_and_barrier(self, tick_clock, wait_clock):
        nc = self.nc
        nop0 = nc.sync.nop(nofuse=True)
        wait_clock.add_sem_waits(nop0.ins, ScopedClock({None: tick_clock.global_clock}))
        si = nop0.ins.sync_info
        waits = list(si.on_wait) if si and si.on_wait else []
        if si and len(waits) > 1:
            del si.on_wait[1:]
            for w in waits[1:]:
                extra = nc.sync.nop(nofuse=True)
                if extra.ins.sync_info is None:
                    extra.ins.sync_info = mybir.SyncInfo(on_wait=[w], on_update=[])
                else:
                    extra.ins.sync_info.on_wait.append(w)
        drain_inst = nc.sync.drain()
        wait_clock.add_sem_waits(
            drain_inst.ins, ScopedClock({None: tick_clock.global_clock})
        )
        dsi = drain_inst.ins.sync_info
        if dsi and dsi.on_wait:
            del dsi.on_wait[:]
        nc.all_engine_barrier()
        popped = nc._tile_sem_poison_stack.pop()
        assert popped is self._sem_poison
        nc.clear_and_free_semaphores(list(self.sems.allocated().values()))
        nc.all_engine_barrier()


def _split_excess_waits(nc: bass.Bass, limit: int = 1) -> None:
    """This walrus build rejects instructions carrying more than ~1 sync
    wait.  Move excess waits onto injected same-engine nops (engines are
    in-order, so gating a preceding nop gates the instruction)."""
    for f in nc.m.functions:
        for bb in f.blocks:
            new_insts = []
            for inst in bb.instructions:
                si = getattr(inst, "sync_info", None)
                if si is not None and si.on_wait and len(si.on_wait) > limit:
                    keep = list(si.on_wait[-limit:])
                    excess = list(si.on_wait[:-limit])
                    for w in excess:
                        nop = mybir.InstNoOp(
                            name=f"I-{nc.next_id()}", engine=inst.engine,
                            ins=[], outs=[],
                            sync_info=mybir.SyncInfo(on_wait=[w], on_update=[]))
                        new_insts.append(nop)
                    del si.on_wait[:]
                    si.on_wait.extend(keep)
                new_insts.append(inst)
            bb.instructions[:] = new_insts


def build_nc(reps: int = 1, debug: bool = False, stop_after: str = 'full') -> bass.Bass:
    nc = bass.Bass("TRN2", target_bir_lowering=False, debug=False,
                   num_devices=N_CORES)

    x_d = nc.dram_tensor("x", [N_NODES, NHID], f32, kind="ExternalInput").ap()
    evi_d = nc.dram_tensor("ev_idx", [NE_LOC, 1], i32, kind="ExternalInput").ap()
    cli_d = nc.dram_tensor("cl_idx", [NC_LOC, 1], i32, kind="ExternalInput").ap()
    cbc_d = nc.dram_tensor("cb_col", [NC_LOC, 1], f32, kind="ExternalInput").ap()
    cbr_d = nc.dram_tensor("cb_row", [1, NC_LOC], f32, kind="ExternalInput").ap()
    ebr_d = nc.dram_tensor("eb_row", [1, NE_LOC], f32, kind="ExternalInput").ap()
    wc_d = nc.dram_tensor("Wc", [NHID, PROJ], f32, kind="ExternalInput").ap()
    bc_d = nc.dram_tensor("bc", [PROJ, 1], f32, kind="ExternalInput").ap()
    we_d = nc.dram_tensor("We", [NHID, PROJ], f32, kind="ExternalInput").ap()
    be_d = nc.dram_tensor("be", [PROJ, 1], f32, kind="ExternalInput").ap()
    wa_d = nc.dram_tensor("Wa", [4 * NHID, NHID], f32, kind="ExternalInput").ap()
    seg_d = nc.dram_tensor("seg", [NG, NHID], f32, kind="ExternalOutput").ap()
    dbg = {}
    if debug:
        dbg["weaug"] = nc.dram_tensor("dbg_weaug", [P, NE_LOC], bf16, kind="ExternalOutput").ap()
        dbg["wcaug"] = nc.dram_tensor("dbg_wcaug", [P, NC_LOC], bf16, kind="ExternalOutput").ap()
        dbg["rowsum"] = nc.dram_tensor("dbg_rowsum", [P, CT], f32, kind="ExternalOutput").ap()
        dbg["cnT"] = nc.dram_tensor("dbg_cnT", [P, NC_LOC], f32, kind="ExternalOutput").ap()
        dbg["aout"] = nc.dram_tensor("dbg_aout", [P, NHID], f32, kind="ExternalOutput").ap()

    with _PatchedTileContext(nc) as tc:
        with (
            tc.tile_pool(name="const", bufs=1) as cpool,
            tc.tile_pool(name="big", bufs=1) as bigpool,
            # PSUM: 2 (transposes + score chunks) + 2 (small matmuls) +
            #       4 (PV accumulators; bank 0 later reused for segsum) = 8
            tc.tile_pool(name="psT", bufs=2, space="PSUM") as psT,
            tc.tile_pool(name="psM", bufs=2, space="PSUM") as psM,
            tc.tile_pool(name="psV", bufs=1, space="PSUM") as psV,
        ):
            # ---------- persistent constants ----------
            ident_b = cpool.tile([P, P], bf16)
            evidx = cpool.tile([P, ET], i32)
            nc.sync.dma_start(evidx[:], evi_d.rearrange("(t p) o -> p (t o)", p=P))
            clidx = cpool.tile([P, CT], i32)
            nc.sync.dma_start(clidx[:], cli_d.rearrange("(t p) o -> p (t o)", p=P))
            cbcol = cpool.tile([P, CT], f32)
            nc.sync.dma_start(cbcol[:], cbc_d.rearrange("(t p) o -> p (t o)", p=P))
            g_col = cpool.tile([PROJ, 1], f32)
            iota_row = cpool.tile([P, NG], f32)
            wc_b = cpool.tile([P, KT_H, PROJ], bf16)
            we_b = cpool.tile([P, KT_H, PROJ], bf16)
            bc_sb = cpool.tile([PROJ, 1], f32)
            nc.sync.dma_start(bc_sb[:], bc_d[:])
            be_sb = cpool.tile([PROJ, 1], f32)
            nc.sync.dma_start(be_sb[:], be_d[:])
            wa_b = cpool.tile([P, KO, NHID], bf16)
            exp_bias = cpool.tile([P, 1], f32)
            nc.gpsimd.memset(exp_bias[:], EXP_BIAS)

            # ---------- persistent big buffers ----------
            ev_bf = bigpool.tile([P, ET, NHID], bf16)      # evidence rows (8 MB)
            p_sb = bigpool.tile([P, CT, NE_LOC], bf16)         # exp'd scores  (8 MB)
            we_aug = bigpool.tile([P, NE_LOC], bf16)           # [64 proj | 64 onehot]
            wc_aug = bigpool.tile([P, NC_LOC], bf16)
            aT = bigpool.tile([P, KO, NC_LOC], bf16)       # concat feats, transposed
            a_out = bigpool.tile([P, CT, NHID], f32)
            oh_seg = bigpool.tile([P, CT, NG], f32)

            # ---------- prologue (scratch freed before the main loop) ----------
            with (
                tc.tile_pool(name="proto1", bufs=1) as pr1,
                tc.tile_pool(name="proto2", bufs=2) as pr2,
            ):
                ident_f = pr1.tile([P, P], f32)
                make_identity(nc, ident_f[:])
                nc.vector.tensor_copy(ident_b[:], ident_f[:])
                ones_row = pr1.tile([1, P], bf16)
                nc.gpsimd.memset(ones_row[:], 1.0)

                g_col_i = pr1.tile([PROJ, 1], i32)
                nc.gpsimd.iota(g_col_i[:], pattern=[[0, 1]], base=0,
                               channel_multiplier=1)
                nc.vector.tensor_copy(g_col[:], g_col_i[:])
                iota_row_i = pr1.tile([P, NG], i32)
                nc.gpsimd.iota(iota_row_i[:], pattern=[[1, NG]], base=0,
                               channel_multiplier=0)
                nc.vector.tensor_copy(iota_row[:], iota_row_i[:])

                cbrow_f = pr1.tile([1, NC_LOC], f32)
                nc.sync.dma_start(cbrow_f[:], cbr_d[:])
                cbrow = pr1.tile([1, NC_LOC], bf16)
                nc.vector.tensor_copy(cbrow[:], cbrow_f[:])

                # projection weights -> bf16 [128, kt, 64]
                for name, dram, out in (("wc", wc_d, wc_b), ("we", we_d, we_b)):
                    stage = pr2.tile([P, KT_H, PROJ], f32, tag="wstage")
                    nc.sync.dma_start(stage[:], dram.rearrange("(k p) m -> p k m", p=P))
                    nc.vector.tensor_copy(out[:], stage[:])
                # Wa -> bf16 [128, 16, 512]: one 4 MB DMA, then convert
                wa_stage = pr1.tile([P, KO, NHID], f32)
                nc.sync.dma_start(wa_stage[:], wa_d.rearrange("(k p) m -> p k m", p=P))
                for k in range(KO):
                    nc.vector.tensor_copy(wa_b[:, k, :], wa_stage[:, k, :])

                # eb/cb broadcast to 64 partitions (ones-matmul), then the
                # one-hot rows of the augmented projections, straight from PSUM
                ebst = pr1.tile([1, NE_LOC], f32)
                nc.sync.dma_start(ebst[:], ebr_d[:])
                ebst_b = pr1.tile([1, NE_LOC], bf16)
                nc.vector.tensor_copy(ebst_b[:], ebst[:])
                for ch in range(NCHUNK):
                    b_ps = psM.tile([PROJ, NHID], f32, tag="mm")
                    nc.tensor.matmul(b_ps[:], ones_row[:, :PROJ],
                                     ebst_b[:, ch * NHID:(ch + 1) * NHID],
                                     start=True, stop=True)
                    nc.vector.tensor_scalar(
                        out=we_aug[PROJ:, ch * NHID:(ch + 1) * NHID], in0=b_ps[:],
                        scalar1=g_col[:], scalar2=MAG, op0=ALU.is_equal,
                        op1=ALU.mult)
                b_ps = psM.tile([PROJ, NHID], f32, tag="mm")
                nc.tensor.matmul(b_ps[:, :NC_LOC], ones_row[:, :PROJ], cbrow[:],
                                 start=True, stop=True)
                nc.vector.tensor_scalar(
                    out=wc_aug[PROJ:, :], in0=b_ps[:, :NC_LOC], scalar1=g_col[:],
                    scalar2=MAG, op0=ALU.is_equal, op1=ALU.mult)
                for t in range(CT):
                    nc.vector.tensor_scalar(
                        out=oh_seg[:, t, :], in0=iota_row[:],
                        scalar1=cbcol[:, t:t + 1], scalar2=None, op0=ALU.is_equal)

            # ---------- main body ----------
            with (
                tc.tile_pool(name="work", bufs=2) as wpool,
                tc.tile_pool(name="ev_ring", bufs=3) as evpool,
                tc.tile_pool(name="pt_ring", bufs=2) as ptpool,
            ):
                for rep in range(reps):
                    if stop_after == 'pro':
                        break
                    # evidence: gather, convert, transpose, project
                    for e in range(ET):
                        ev_f = evpool.tile([P, NHID], f32, tag="ev_f")
                        nc.gpsimd.indirect_dma_start(
                            out=ev_f[:], out_offset=None, in_=x_d[:],
                            in_offset=bass.IndirectOffsetOnAxis(
                                ap=evidx[:, e:e + 1], axis=0))
                        nc.vector.tensor_copy(ev_bf[:, e, :], ev_f[:])
                        we_ps = psM.tile([PROJ, P], f32, tag="mm")
                        for k in range(KT_H):
                            tp_ps = psT.tile([P, P], bf16, tag="tp")
                            nc.tensor.transpose(
                                tp_ps[:], ev_bf[:, e, k * P:(k + 1) * P], ident_b[:])
                            evT = evpool.tile([P, P], bf16, tag="evT")
                            nc.scalar.copy(evT[:], tp_ps[:])
                            nc.tensor.matmul(we_ps[:], we_b[:, k, :], evT[:],
                                             start=(k == 0), stop=(k == KT_H - 1))
                        nc.scalar.activation(we_aug[:PROJ, e * P:(e + 1) * P],
                                             we_ps[:], AF.Identity, bias=be_sb[:])

                    if stop_after == 'ev':
                        break
                    # claims: gather, convert, transpose into aT[0:4], project
                    for t in range(CT):
                        cl_f = wpool.tile([P, NHID], f32, tag="cl_f")
                        nc.gpsimd.indirect_dma_start(
                            out=cl_f[:], out_offset=None, in_=x_d[:],
                            in_offset=bass.IndirectOffsetOnAxis(
                                ap=clidx[:, t:t + 1], axis=0))
                        cl_b = wpool.tile([P, NHID], bf16, tag="cl_b")
                        nc.vector.tensor_copy(cl_b[:], cl_f[:])
                        for k in range(KT_H):
                            tp_ps = psT.tile([P, P], bf16, tag="tp")
                            nc.tensor.transpose(tp_ps[:], cl_b[:, k * P:(k + 1) * P],
                                                ident_b[:])
                            nc.vector.tensor_copy(aT[:, k, t * P:(t + 1) * P],
                                                  tp_ps[:])
                    for t in range(CT):
                        wcp = psM.tile([PROJ, P], f32, tag="mm")
                        for k in range(KT_H):
                            nc.tensor.matmul(wcp[:], wc_b[:, k, :],
                                             aT[:, k, t * P:(t + 1) * P],
                                             start=(k == 0), stop=(k == KT_H - 1))
                        nc.scalar.activation(wc_aug[:PROJ, t * P:(t + 1) * P],
                                             wcp[:], AF.Identity, bias=bc_sb[:])

                    if stop_after == 'claims':
                        break
                    # scores + masked softmax with fixed exp shift
                    for t in range(CT):
                        parts = wpool.tile([P, NCHUNK], f32, tag="parts")
                        for ch in range(NCHUNK):
                            s_ps = psT.tile([P, NHID], f32, tag="tp")
                            nc.tensor.matmul(s_ps[:], wc_aug[:, t * P:(t + 1) * P],
                                             we_aug[:, ch * NHID:(ch + 1) * NHID],
                                             start=True, stop=True)
                            nc.scalar.activation(
                                p_sb[:, t, ch * NHID:(ch + 1) * NHID], s_ps[:],
                                AF.Exp, bias=exp_bias[:],
                                accum_out=parts[:, ch:ch + 1])
                        rowsum = wpool.tile([P, 1], f32, tag="rowsum")
                        nc.vector.reduce_sum(rowsum[:], parts[:],
                                             axis=mybir.AxisListType.X)
                        if debug and rep == 0:
                            nc.sync.dma_start(dbg["rowsum"][:, t:t + 1], rowsum[:])
                        recip = wpool.tile([P, 1], f32, tag="recip")
                        nc.vector.reciprocal(recip[:], rowsum[:])
                        nc.vector.tensor_scalar(
                            out=p_sb[:, t, :], in0=p_sb[:, t, :], scalar1=recip[:],
                            scalar2=None, op0=ALU.mult)

                    if stop_after == 'scores':
                        break
                    # PV: cnT[h] = sum_e ev[e, h].T @ P[e]
                    pv_ps = [psV.tile([P, NC_LOC], f32, tag=f"pv{h}",
                                      name=f"pv{h}") for h in range(KT_H)]
                    for e in range(ET):
                        pt = ptpool.tile([P, NC_LOC], bf16, tag="pt")
                        for t in range(CT):
                            tp_ps = psT.tile([P, P], bf16, tag="tp",
                                             name=f"tp{e}_{t}")
                            nc.tensor.transpose(tp_ps[:],
                                                p_sb[:, t, e * P:(e + 1) * P],
                                                ident_b[:])
                            if t % 2 == 0:
                                nc.scalar.copy(pt[:, t * P:(t + 1) * P], tp_ps[:])
                            else:
                                nc.vector.tensor_copy(pt[:, t * P:(t + 1) * P],
                                                      tp_ps[:])
                        for h in range(KT_H):
                            nc.tensor.matmul(pv_ps[h][:],
                                             ev_bf[:, e, h * P:(h + 1) * P], pt[:],
                                             start=(e == 0), stop=(e == ET - 1))

                    # aT blocks 4..15: cn, claim-cn, claim*cn (all transposed)
                    for h in range(KT_H):
                        nc.scalar.copy(aT[:, KT_H + h, :], pv_ps[h][:])
                        nc.vector.tensor_tensor(
                            out=aT[:, 2 * KT_H + h, :], in0=aT[:, h, :],
                            in1=pv_ps[h][:], op=ALU.subtract)
                        nc.vector.tensor_tensor(
                            out=aT[:, 3 * KT_H + h, :], in0=aT[:, h, :],
                            in1=pv_ps[h][:], op=ALU.mult)
                    if debug and rep == 0:
                        cnT_f = wpool.tile([P, NC_LOC], f32, tag="dbgcn")
                        nc.vector.tensor_copy(cnT_f[:], pv_ps[0][:])
                        nc.sync.dma_start(dbg["cnT"][:], cnT_f[:])

                    if stop_after == 'pv':
                        break
                    # a @ Wa (ba added on host)
                    for t in range(CT):
                        o_ps = psM.tile([P, NHID], f32, tag="mm")
                        for k in range(KO):
                            nc.tensor.matmul(o_ps[:], aT[:, k, t * P:(t + 1) * P],
                                             wa_b[:, k, :], start=(k == 0),
                                             stop=(k == KO - 1))
                        nc.scalar.copy(a_out[:, t, :], o_ps[:])

                    # segment sum via one-hot matmul (fp32)
                    seg_ps = psV.tile([NG, NHID], f32, tag="pv0")
                    for t in range(CT):
                        nc.tensor.matmul(seg_ps[:], oh_seg[:, t, :], a_out[:, t, :],
                                         start=(t == 0), stop=(t == CT - 1))
                    seg_sb = wpool.tile([NG, NHID], f32, tag="seg_sb")
                    nc.scalar.copy(seg_sb[:], seg_ps[:])
                    nc.sync.dma_start(seg_d[:], seg_sb[:])

                    if debug and rep == 0:
                        nc.sync.dma_start(dbg["weaug"][:], we_aug[:])
                        nc.sync.dma_start(dbg["wcaug"][:], wc_aug[:])
                        nc.sync.dma_start(dbg["aout"][:], a_out[:, 0, :])
    _split_excess_waits(nc)
    return nc


def make_in_maps(inputs: dict) -> tuple[list[dict], np.ndarray, np.ndarray]:
    """Host-side index preprocessing + per-core input maps."""
    batch = np.asarray(inputs["batch"]).astype(np.int64)
    ci = np.asarray(inputs["claim_index"]).astype(np.int64)
    ei = np.asarray(inputs["evidence_index"]).astype(np.int64)
    x = np.ascontiguousarray(np.asarray(inputs["x"], dtype=np.float32))
    cb = batch[ci].astype(np.float32)
    eb = batch[ei].astype(np.float32)
    counts = np.bincount(batch[ci], minlength=NG).astype(np.float32)
    ba = np.asarray(inputs["ba"], dtype=np.float32).reshape(NHID)

    order = np.argsort(cb, kind="stable")
    ci, cb = ci[order], cb[order]

    common = {
        "x": x,
        "Wc": np.ascontiguousarray(np.asarray(inputs["Wc"], dtype=np.float32)),
        "bc": np.asarray(inputs["bc"], dtype=np.float32).reshape(PROJ, 1),
        "We": np.ascontiguousarray(np.asarray(inputs["We"], dtype=np.float32)),
        "be": np.asarray(inputs["be"], dtype=np.float32).reshape(PROJ, 1),
        "Wa": np.ascontiguousarray(np.asarray(inputs["Wa"], dtype=np.float32)),
    }
    in_maps = []
    for c in range(N_CORES):
        sl = slice(c * NC_LOC, (c + 1) * NC_LOC)
        m = dict(common)
        m["cl_idx"] = ci[sl].astype(np.int32).reshape(NC_LOC, 1)
        m["cb_col"] = cb[sl].reshape(NC_LOC, 1)
        m["cb_row"] = cb[sl].reshape(1, NC_LOC)
        # evidence restricted to this core's graphs, padded to NE_LOC with
        # graph id -2 (matches nothing -> exactly-zero attention weight)
        sel = np.where(np.isin(eb, np.unique(cb[sl])))[0]
        assert len(sel) <= NE_LOC, f"core {c}: {len(sel)} evidence rows > {NE_LOC}"
        ev_c = np.zeros(NE_LOC, np.int32)
        eb_c = np.full(NE_LOC, -2.0, np.float32)
        ev_c[:len(sel)] = ei[sel]
        eb_c[:len(sel)] = eb[sel]
        m["ev_idx"] = ev_c.reshape(NE_LOC, 1)
        m["eb_row"] = eb_c.reshape(1, NE_LOC)
        in_maps.append(m)
    return in_maps, counts, ba


def postprocess(results: list, counts: np.ndarray, ba: np.ndarray) -> np.ndarray:
    seg = np.zeros((NG, NHID), np.float64)
    for c in range(N_CORES):
        seg += results[c]["seg"].astype(np.float64)
    # segment_mean(a + ba) = segment_mean(a) + ba, except empty graphs stay 0
    out = seg / np.maximum(counts, 1.0)[:, None] + (counts > 0)[:, None] * ba[None, :]
    return out.astype(np.float32)


def kernel(**inputs) -> np.ndarray:
    nc = build_nc()
    in_maps, counts, ba = make_in_maps(inputs)
    res = run_bass_kernel_spmd(nc, in_maps, list(range(N_CORES)))
    return postprocess(res.results, counts, ba)

